# revision 74
# baseline (speedup 1.0000x reference)
"""Trainium2 Bass kernel for nn_MultiHeadAttn_80126909874682.

Full MHA layer: QKV projection -> 16-head attention (seq 2048) -> output
projection -> residual -> LayerNorm, over h [2048, 4, 1024] fp32.

Sharding (8 NeuronCores, zero collectives):
  core c -> batch b = c // 2, token-half r = c % 2.
  Each core computes K/V for all 2048 tokens of its batch (all 16 heads)
  and Q / attention / output projection / LayerNorm for its 1024 local
  tokens only.  The per-core `hb` input is permuted so the core's local
  tokens come first; attention is invariant to the j-permutation of K/V.

v7: fp8 DoubleRow matmuls + per-head two-engine softmax.
  - Projection / score / output GEMMs run in fp8 e4m3 with
    MatmulPerfMode.DoubleRow (2 k-tiles per instruction at 0.5
    cycles/output-row).  Weights and h^T are pre-interleaved on the
    host in [128, 2, N] k-pair layout and upscaled x32 so e4m3 has
    mantissa headroom; the scale is folded into the exp argument
    (1/16384, which also absorbs the stride-0 score doubling) and the
    output-projection epilogue (1/1024).
  - Score matmuls (contraction d_head=64) use DoubleRow with a
    stride-0 broadcast on dim1 of both operands: the PE computes
    2*K^T Q at half cost; the doubling is folded into the exp scale.
  - Softmax exp is split per head so ACT and DVE run concurrently
    within each pair: even heads use the ACT Exp activation writing
    fp8 e5m2 probs, consumed by fp8 DoubleRow PV matmuls; odd heads
    use the DVE Schraudolph bit-trick (int16(x*1477.32/16384 + 15315)
    bitcast to fp16 ~= exp(x)) with plain fp16 PV.  V is produced in the matching
    dtype per head via a host-side Wv column permutation that groups
    each engine's heads contiguously.  A ones-column in V makes the PV
    matmul emit softmax denominators.  PV trails the exp stream by one
    jg so the PE never head-of-line blocks on an exp result.
  - GPSIMD cannot touch PSUM, so PSUM->SBUF casts go to ACT (K/Q/V)
    and DVE (prob divides, residual); Pool keeps the SBUF-only work
    (partition broadcasts, LayerNorm normalize, DMAs).  The whole
    (pair, itile, jg) slot sequence is software-pipelined: PVs trail
    the exp stream by two slots across itile boundaries, divide chains
    chase the last PV, and projection / V-production matmul groups for
    upcoming pairs fill the PE slack inside the attention slots.
  - LayerNorm applies (x - mean) * rstd on device; the gamma/beta
    affine is applied on the host after gathering (exact for any
    gamma/beta).  One act-table load (ln+exp set) is emitted up front.
"""

import os
import sys

os.environ.setdefault("JAX_PLATFORMS", "axon")
sys.path.insert(0, "/opt/trn_rl_repo")

import numpy as np
import ml_dtypes

import concourse.bass as bass
import concourse.tile as tile
from concourse import bacc, mybir
from concourse.bass import ts
from concourse.bass_utils import run_bass_kernel_spmd
from concourse.hw_specs import get_activation_tables

N_HEAD = 16
D_MODEL = 1024
D_HEAD = 64
SEQ = 2048
BATCH = 4
EPS = 1e-5
N_CORES = 8

LOCAL = SEQ // 2            # tokens owned per core (1024)
N_PAIR = N_HEAD // 2        # head pairs (8)
CC = D_MODEL // 128         # 128-contraction chunks (8)
CP = CC // 2                # DoubleRow contraction pair chunks (4)
JT = SEQ // 128             # j tiles (16)
JG = JT // 2                # j tile pairs (8)
IB_ALL = SEQ // 512         # 512-token blocks, all tokens (4)
IB_LOC = LOCAL // 512       # 512-token blocks, local tokens (2)

F32 = mybir.dt.float32
F16 = mybir.dt.float16
I16 = mybir.dt.int16
E4 = mybir.dt.float8e4
E5 = mybir.dt.float8e5
AF = mybir.ActivationFunctionType
ALU = mybir.AluOpType
DR = mybir.MatmulPerfMode.DoubleRow

UPS = 32.0                                  # host weight upscale
SCALE_S = 1.0 / (2.0 * UPS * UPS * 8.0)     # exp scale: stride0 x2, x32 q/k, sqrt(64)
INV_OUT = 1.0 / (UPS * UPS)                 # out-proj downscale
A16 = 1024.0 / float(np.log(2.0))           # Schraudolph fp16 slope
C16 = 15360.0 - 45.0                        # Schraudolph fp16 offset

FLEX_MOD = 10      # every FLEX_MOD-th odd-head exp slot runs on ACT
FLEX_PHASE = 5
# Heads whose exp runs on DVE (Schraudolph, fp16 probs): odd heads.
DVE_HEADS = frozenset((1, 3, 5, 7, 9, 11, 13, 15))
# Per half: fp8 heads first, then fp16 heads (host permutes Wv columns).
V_ORDER = [[n for n in range(8) if n not in DVE_HEADS]
           + [n for n in range(8) if n in DVE_HEADS],
           [n for n in range(8, 16) if n not in DVE_HEADS]
           + [n for n in range(8, 16) if n in DVE_HEADS]]
N_V8 = [sum(1 for n in V_ORDER[0] if n not in DVE_HEADS),
        sum(1 for n in V_ORDER[1] if n not in DVE_HEADS)]
VIDX = {}
for _half in range(2):
    for _i, _n in enumerate(V_ORDER[_half]):
        VIDX[_n] = _i if _i < N_V8[_half] else _i - N_V8[_half]


def build_program():
    nc = bacc.Bacc()

    hb = nc.declare_dram_parameter("hb", [LOCAL, D_MODEL], F32, isOutput=False)
    hbt_d = nc.declare_dram_parameter("hbt", [512, 2 * SEQ], E4, isOutput=False)
    wq = nc.declare_dram_parameter("wq", [512, 2 * D_MODEL], E4, isOutput=False)
    wk = nc.declare_dram_parameter("wk", [512, 2 * D_MODEL], E4, isOutput=False)
    wv = nc.declare_dram_parameter("wv", [512, 2 * D_MODEL], E4, isOutput=False)
    wo = nc.declare_dram_parameter("wo", [512, 2 * D_MODEL], E4, isOutput=False)
    out = nc.declare_dram_parameter("out", [LOCAL, D_MODEL], F32, isOutput=True)

    with tile.TileContext(nc) as tc:
        with (
            tc.tile_pool(name="consts", bufs=1) as consts,
            tc.tile_pool(name="wo_w", bufs=1) as wo_pool,
            tc.tile_pool(name="hbt", bufs=1) as hbt_pool,
            tc.tile_pool(name="w_qk", bufs=1) as wqk_pool,
            tc.tile_pool(name="w_v", bufs=1) as wv_pool,
            tc.tile_pool(name="vsb", bufs=1) as v_pool,
            tc.tile_pool(name="ktq", bufs=3) as ktq_pool,
            tc.tile_pool(name="attnT", bufs=1) as attn_pool,
            tc.tile_pool(name="exp5", bufs=9) as e5_pool,
            tc.tile_pool(name="exp16", bufs=8) as e16_pool,
            tc.tile_pool(name="small", bufs=2) as rec_pool,
            tc.tile_pool(name="xstage", bufs=3) as x_pool,
            tc.tile_pool(name="hbres", bufs=1) as hbr_pool,
            tc.tile_pool(name="stage", bufs=6) as stg_pool,
            tc.tile_pool(name="psum1", bufs=1, space="PSUM") as psum1,
            tc.tile_pool(name="psum3", bufs=3, space="PSUM") as psum2,
        ):
            _emit(nc, tc, hb, hbt_d, wq, wk, wv, wo, out,
                  consts, wo_pool, hbt_pool, wqk_pool, wv_pool, v_pool,
                  ktq_pool, attn_pool, e5_pool, e16_pool, rec_pool, x_pool,
                  hbr_pool, stg_pool, psum1, psum2)

    nc.finalize()
    return nc


def _emit(nc, tc, hb, hbt_d, wq, wk, wv, wo, out,
          consts, wo_pool, hbt_pool, wqk_pool, wv_pool, v_pool,
          ktq_pool, attn_pool, e5_pool, e16_pool, rec_pool, x_pool,
          hbr_pool, stg_pool, psum1, psum2):
    # ---- one act-table load covering Exp + Ln (avoids reload churn) ----
    tables = get_activation_tables(nc.m.arch)
    set_id = list(tables).index("natural_log_exp_and_others")
    nc.scalar.add_instruction(mybir.InstLoadActFuncSet(
        name=nc.get_next_instruction_name(), ins=[], outs=[],
        act_func_set_id=set_id))

    # ---- constants ----
    eps_t = consts.tile([128, 1], F32)
    nc.vector.memset(eps_t[:], EPS)

    # ---- SBUF weight tiles (DoubleRow k-pair interleaved) ----
    wo_sb = [wo_pool.tile([128, 2, D_MODEL], E4, tag=f"wo{c}", name=f"wo{c}")
             for c in range(CP)]
    wq_sb = [wqk_pool.tile([128, 2, D_MODEL], E4, tag=f"wq{c}", name=f"wq{c}")
             for c in range(CP)]
    wk_sb = [wqk_pool.tile([128, 2, D_MODEL], E4, tag=f"wk{c}", name=f"wk{c}")
             for c in range(CP)]

    # V tiles: [128 j, 16 jc, nh, 65] with ones column at d=64; fp8 heads
    # first then fp16 heads per half (host permutes Wv columns to match).
    v8 = [v_pool.tile([128, N_V8[hf], JT, 128], E4, tag=f"v8_{hf}", name=f"v8_{hf}")
          for hf in range(2)]
    v16 = [v_pool.tile([128, max(1, 8 - N_V8[hf]), JT, 68], F16,
                       tag=f"v16_{hf}", name=f"v16_{hf}")
           for hf in range(2)]
    for t in (*v8, *v16):
        nc.vector.memset(t[:, :, :, 64:65], 1.0)

    # ---- h^T fp8 DR tiles + residual rows (preloaded) ----
    hbt = [hbt_pool.tile([128, 2, SEQ], E4, tag=f"hbt{c}", name=f"hbt{c}")
           for c in range(CP)]
    for c, eng in zip(range(CP), (nc.sync, nc.gpsimd, nc.scalar, nc.sync)):
        eng.dma_start(hbt[c][:], hbt_d[ts(c, 128), :])
    hbres = [hbr_pool.tile([128, D_MODEL], F32, tag=f"hbres{i}", name=f"hbres{i}")
             for i in range(8)]
    for i in range(8):
        nc.sync.dma_start(hbres[i][:], hb[ts(i, 128), :])

    def v_dma(half):
        wv_sb = [wv_pool.tile([128, 2, 512], E4, tag=f"wv{half}_{c}",
                              name=f"wv{half}_{c}")
                 for c in range(CP)]
        for c in range(CP):
            nc.gpsimd.dma_start(
                wv_sb[c][:],
                wv[ts(c, 128), 2 * 512 * half: 2 * 512 * (half + 1)])
        return wv_sb

    def v_group(half, wv_sb, j):
        """Produce V (+ones) for one j-chunk of heads 8*half..8*half+7."""
        n8 = N_V8[half]
        ps = psum2.tile([128, 1024], F32, tag="s2", name="vps")[:, 0:512]
        for c in range(CP):
            nc.tensor.matmul(
                ps[:], hbt[c][:, :, ts(j, 128)], wv_sb[c][:],
                start=(c == 0), stop=(c == CP - 1), perf_mode=DR,
            )
        src = ps[:].rearrange("p (n d) -> p n d", n=8)
        nc.scalar.copy(
            v8[half][:, :, j:j + 1, 0:64].squeeze(2), src[:, 0:n8, :])
        if n8 < 8:
            nc.scalar.copy(
                v16[half][:, :, j:j + 1, 0:64].squeeze(2), src[:, n8:8, :])

    # attn output, DoubleRow pair-interleaved for the output projection:
    # at[(pp, itile)] = [128 nd-in-pair, 4 isub, 2 pair-slot, 128 i] fp8
    at = {}
    for pp in range(N_PAIR // 2):
        for itile in range(IB_LOC):
            at[(pp, itile)] = attn_pool.tile(
                [128, 4, 2, 128], E4, tag=f"at{pp}_{itile}", name=f"at{pp}_{itile}")

    def wo_block(itile, pool_heavy=False):
        for s4 in range(4):
            wo_isub(itile, s4, pool_heavy)

    def wo_isub(itile, s4, pool_heavy=False):
        if True:
            isub = 4 * itile + s4
            x = x_pool.tile([128, D_MODEL], F32, tag="x", name="x")
            opsb = psum2.tile([128, 1024], F32, tag="s2", name="ops")
            for dm in range(2):
                ops = opsb[:, ts(dm, 512)]
                for pp in range(N_PAIR // 2):
                    nc.tensor.matmul(
                        ops, at[(pp, itile)][:, s4:s4 + 1, :, :].squeeze(1),
                        wo_sb[pp][:, :, ts(dm, 512)],
                        start=(pp == 0), stop=(pp == N_PAIR // 2 - 1),
                        perf_mode=DR,
                    )
                # x = ops * (1/1024) + hbres   (undo weight upscale)
                nc.vector.scalar_tensor_tensor(
                    x[:, ts(dm, 512)], ops, INV_OUT,
                    hbres[isub][:, ts(dm, 512)],
                    op0=ALU.mult, op1=ALU.add,
                )
            stats = rec_pool.tile([128, 2, 6], F32, tag="bnst", name="st")
            mv = rec_pool.tile([128, 2], F32, tag="bnmv", name="mv")
            for g in range(2):
                nc.vector.bn_stats(stats[:, g, :], x[:, ts(g, 512)])
            nc.vector.bn_aggr(mv[:], stats[:])
            rstd = rec_pool.tile([128, 1], F32, tag="rstd", name="rstd")
            nc.scalar.activation(rstd[:], mv[:, 1:2], AF.Ln, bias=eps_t[:])
            nc.scalar.activation(rstd[:], rstd[:], AF.Exp, scale=-0.5)
            y = x_pool.tile([128, D_MODEL], F32, tag="y", name="y")
            nc.gpsimd.tensor_scalar(
                y[:], x[:], mv[:, 0:1], rstd[:],
                op0=ALU.subtract, op1=ALU.mult,
            )
            nc.sync.dma_start(out[ts(isub, 128), :], y[:])

    def recip_of(acc, h):
        rec = rec_pool.tile([1, 512], F32, tag="rec", name="rec")
        nc.vector.reciprocal(rec[:], acc[h][64:65, :])
        return rec

    def divide(p, itile, acc, h, rec):
        """Normalize acc -> at (broadcast on Pool, divide DVE/Pool)."""
        rb = rec_pool.tile([64, 512], F32, tag="recb", name="rb")
        nc.gpsimd.partition_broadcast(rb[:], rec[:])
        pp, slot = divmod(p, 2)
        dst = at[(pp, itile)][ts(h, 64), :, slot:slot + 1, :].squeeze(2)
        nc.vector.tensor_tensor(
            dst,
            acc[h][0:64, :].rearrange("p (a b) -> p a b", a=4),
            rb[:].rearrange("p (a b) -> p a b", a=4),
            op=ALU.mult,
        )

    def kq_group(kt_p, qt_p, p, g):
        """One projection matmul group for pair p (g<4: K j-block, else Q)."""
        if g < IB_ALL:
            ps = psum2.tile([128, 1024], F32, tag="s2", name="kps")[:, 0:512]
            for c in range(CP):
                nc.tensor.matmul(
                    ps[:], wk_sb[c][:, :, ts(p, 128)], hbt[c][:, :, ts(g, 512)],
                    start=(c == 0), stop=(c == CP - 1), perf_mode=DR,
                )
            nc.scalar.copy(kt_p[:, ts(g, 512)], ps[:])
        else:
            ib = g - IB_ALL
            ps = psum2.tile([128, 1024], F32, tag="s2", name="qps")[:, 0:512]
            for c in range(CP):
                nc.tensor.matmul(
                    ps[:], wq_sb[c][:, :, ts(p, 128)], hbt[c][:, :, ts(ib, 512)],
                    start=(c == 0), stop=(c == CP - 1), perf_mode=DR,
                )
            nc.scalar.copy(qt_p[:, ts(ib, 512)], ps[:])

    ktq = {}  # pair -> (kt, qt) tiles (ring of 2)

    def new_ktq(p):
        ktq[p] = (ktq_pool.tile([128, SEQ], E4, tag="kt", name="kt_p"),
                  ktq_pool.tile([128, LOCAL], E4, tag="qt", name="qt_p"))
        return ktq[p]

    fillers = []  # deferred emission closures, popped 2 per jg slot

    # pair 0 prologue: weights, own projections up front, V fillers.
    # wq first and Q-projection before K: the first scores need qt(it0)
    # plus only the first K j-block.
    for c in range(CP):
        nc.gpsimd.dma_start(wq_sb[c][:], wq[ts(c, 128), :])
        nc.scalar.dma_start(wk_sb[c][:], wk[ts(c, 128), :])
        nc.sync.dma_start(wo_sb[c][:], wo[ts(c, 128), :])
    kt0, qt0 = new_ktq(0)
    for g in (IB_ALL, 0, IB_ALL + 1, 1, 2, 3):
        kq_group(kt0, qt0, 0, g)
    wv0 = v_dma(0)
    fillers += [(lambda half=0, sb=wv0, j=j: v_group(half, sb, j))
                for j in range(JT)]

    def pv(p, h, jg, acc, es):
        half = p // 4
        n = 2 * p + h
        e = es[(jg, h)]
        if n in DVE_HEADS:
            ef = e[:].bitcast(F16) if e.dtype == I16 else e[:]
            for u in range(2):
                jc = 2 * jg + u
                nc.tensor.matmul(
                    acc[h][0:65, :],
                    v16[half][:, VIDX[n]:VIDX[n] + 1, jc:jc + 1, 0:65]
                    .squeeze(1).squeeze(1),
                    ef[:, ts(u, 512)],
                    start=(jg == 0 and u == 0),
                    stop=(jg == JG - 1 and u == 1),
                )
        else:
            nc.tensor.matmul(
                acc[h][:],
                v8[half][:, VIDX[n]:VIDX[n] + 1, 2 * jg:2 * jg + 2, :]
                .squeeze(1),
                e[:].rearrange("p (a b) -> p a b", a=2),
                start=(jg == 0), stop=(jg == JG - 1),
                perf_mode=DR,
            )

    # Global slot pipeline: per (p, itile, jg) slot emit scores+exp, pop
    # one trailing PV (previous slot's, possibly across itile/pair
    # boundaries), then fillers.  When an itile's last PV retires, its
    # divide chain (and wo_block(0) for pair 7 itile 0) follows at once.
    pvq = []     # FIFO of (closure, tail_action or None)

    def drain_one():
        if pvq:
            clo, tail = pvq.pop(0)
            clo()
            if tail is not None:
                tail()

    for p in range(N_PAIR):
        if p == 3:
            wv1 = v_dma(1)
            fillers += [(lambda half=1, sb=wv1, j=j: v_group(half, sb, j))
                        for j in range(JT)]

        kt_p, qt_p = ktq.pop(p)

        for itile in range(IB_LOC):
            if itile == 0 and p < N_PAIR - 1:
                ktn, qtn = new_ktq(p + 1)
                fillers += [(lambda k=ktn, q=qtn, pn=p + 1, g=g:
                             kq_group(k, q, pn, g))
                            for g in range(IB_ALL + IB_LOC)]
            acc = [psum1.tile([128, 512], F32, tag=f"acc{h}", name="acc")
                   for h in range(2)]
            es = {}
            for jg in range(JG):
                for h in range(2):
                    s2 = psum2.tile([128, 1024], F32, tag="s2", name="s2")
                    for u in range(2):
                        jc = 2 * jg + u
                        lhsT = (kt_p[ts(h, 64), ts(jc, 128)]
                                .unsqueeze(1).broadcast_to([64, 2, 128]))
                        rhs = (qt_p[ts(h, 64), ts(itile, 512)]
                               .unsqueeze(1).broadcast_to([64, 2, 512]))
                        nc.tensor.matmul(
                            s2[:, ts(u, 512)], lhsT, rhs,
                            start=True, stop=True, perf_mode=DR,
                        )
                    if (2 * p + h) in DVE_HEADS:
                        slot = (2 * p + itile) * 8 + jg
                        if slot % FLEX_MOD == FLEX_PHASE:
                            # flex this odd-head exp onto ACT (fp16 out,
                            # same fp16 PV path) to balance engine load
                            e = e16_pool.tile([128, 1024], F16, tag="e16",
                                              name="e16")
                            nc.scalar.activation(e[:], s2[:], AF.Exp,
                                                 scale=SCALE_S)
                        else:
                            e = e16_pool.tile([128, 1024], I16, tag="e16",
                                              name="e16")
                            nc.vector.tensor_scalar(
                                e[:], s2[:], A16 * SCALE_S, C16,
                                op0=ALU.mult, op1=ALU.add,
                            )
                    else:
                        e = e5_pool.tile([128, 1024], E5, tag="e5", name="e5")
                        nc.scalar.activation(e[:], s2[:], AF.Exp, scale=SCALE_S)
                    es[(jg, h)] = e

                def tail(p=p, itile=itile, acc=acc, es=es):
                    recs = [recip_of(acc, h) for h in range(2)]
                    for h in range(2):
                        divide(p, itile, acc, h, recs[h])
                    if p == N_PAIR - 1 and itile == 0:
                        for s4 in range(4):
                            fillers.extend(
                                [(lambda s4=s4: wo_isub(0, s4,
                                                        pool_heavy=True)),
                                 (lambda: None), (lambda: None),
                                 (lambda: None)])

                if len(pvq) > 3:
                    drain_one()
                pvq.append((
                    (lambda p=p, jg=jg, acc=acc, es=es:
                     [pv(p, h, jg, acc, es) for h in range(2)]),
                    tail if jg == JG - 1 else None,
                ))
                for _ in range(2):
                    if fillers:
                        fillers.pop(0)()
    while pvq:
        drain_one()
    while fillers:
        fillers.pop(0)()
    wo_block(1)


_program_cache = {}


def _get_program():
    if "nc" not in _program_cache:
        _program_cache["nc"] = build_program()
    return _program_cache["nc"]


def _interleave_k(w):
    """[1024, C] -> [512, 2C] DoubleRow k-pair layout: out[128*cp + k,
    t*C + c] = w[256*cp + 128*t + k, c]."""
    C = w.shape[1]
    return np.ascontiguousarray(
        w.reshape(4, 2, 128, C).transpose(0, 2, 1, 3).reshape(512, 2 * C))


def _shard_inputs(h, Wq, Wkv, Wo):
    """Build the 8 per-core input maps (host-side numpy only)."""
    h = np.asarray(h, np.float32)
    Wq = np.asarray(Wq, np.float32)
    Wkv = np.asarray(Wkv, np.float32)
    Wo = np.asarray(Wo, np.float32)

    E4n = ml_dtypes.float8_e4m3
    Wq8 = _interleave_k((Wq * UPS).astype(E4n))
    Wk8 = _interleave_k((Wkv[:, :N_HEAD * D_HEAD] * UPS).astype(E4n))
    Wv = Wkv[:, N_HEAD * D_HEAD:] * UPS
    # permute V columns: per half, fp8 heads first then fp16 heads
    perm = [n * 64 + d for hf in range(2) for n in V_ORDER[hf]
            for d in range(64)]
    Wvp = np.ascontiguousarray(Wv[:, perm]).astype(E4n)
    # interleave each 512-column half separately so the per-half DMA
    # slice is a valid [128, 2, 512] DoubleRow tile
    Wv8 = np.concatenate(
        [_interleave_k(Wvp[:, 512 * hf:512 * (hf + 1)]) for hf in range(2)],
        axis=1)
    Wo8 = _interleave_k((Wo * UPS).astype(E4n))

    in_maps = []
    for core in range(N_CORES):
        b, r = divmod(core, 2)
        hb_full = h[:, b, :]  # [2048, 1024]
        if r == 0:
            hb_perm = hb_full
        else:
            hb_perm = np.concatenate([hb_full[LOCAL:], hb_full[:LOCAL]], axis=0)
        hbt8 = _interleave_k(np.ascontiguousarray(hb_perm.T).astype(E4n))
        in_maps.append({
            "hb": np.ascontiguousarray(hb_perm[:LOCAL]),
            "hbt": hbt8,
            "wq": Wq8, "wk": Wk8, "wv": Wv8, "wo": Wo8,
        })
    return in_maps


def kernel(h, Wq, Wkv, Wo, gamma, beta, _trace=False):
    nc = _get_program()
    in_maps = _shard_inputs(h, Wq, Wkv, Wo)
    res = run_bass_kernel_spmd(nc, in_maps, list(range(N_CORES)), trace=_trace)
    if _trace:
        kernel.last_results = res

    gamma = np.asarray(gamma, np.float32)
    beta = np.asarray(beta, np.float32)
    out = np.empty((SEQ, BATCH, D_MODEL), np.float32)
    for core in range(N_CORES):
        b, r = divmod(core, 2)
        out[r * LOCAL:(r + 1) * LOCAL, b, :] = res.results[core]["out"]
    # LayerNorm gamma/beta affine (device emits (x - mean) * rstd)
    out *= gamma
    out += beta
    return out


# revision 86
# speedup vs baseline: 1.0010x; 1.0010x over previous
"""Trainium2 Bass kernel for nn_MultiHeadAttn_80126909874682.

Full MHA layer: QKV projection -> 16-head attention (seq 2048) -> output
projection -> residual -> LayerNorm, over h [2048, 4, 1024] fp32.

Sharding (8 NeuronCores, zero collectives):
  core c -> batch b = c // 2, token-half r = c % 2.
  Each core computes K/V for all 2048 tokens of its batch (all 16 heads)
  and Q / attention / output projection / LayerNorm for its 1024 local
  tokens only.  The per-core `hb` input is permuted so the core's local
  tokens come first; attention is invariant to the j-permutation of K/V.

v7: fp8 DoubleRow matmuls + per-head two-engine softmax.
  - Projection / score / output GEMMs run in fp8 e4m3 with
    MatmulPerfMode.DoubleRow (2 k-tiles per instruction at 0.5
    cycles/output-row).  Weights and h^T are pre-interleaved on the
    host in [128, 2, N] k-pair layout and upscaled x32 so e4m3 has
    mantissa headroom; the scale is folded into the exp argument
    (1/16384, which also absorbs the stride-0 score doubling) and the
    output-projection epilogue (1/1024).
  - Score matmuls (contraction d_head=64) use DoubleRow with a
    stride-0 broadcast on dim1 of both operands: the PE computes
    2*K^T Q at half cost; the doubling is folded into the exp scale.
  - Softmax exp is split per head so ACT and DVE run concurrently
    within each pair: even heads use the ACT Exp activation writing
    fp8 e5m2 probs, consumed by fp8 DoubleRow PV matmuls; odd heads
    use the DVE Schraudolph bit-trick (int16(x*1477.32/16384 + 15315)
    bitcast to fp16 ~= exp(x)) with plain fp16 PV.  V is produced in the matching
    dtype per head via a host-side Wv column permutation that groups
    each engine's heads contiguously.  A ones-column in V makes the PV
    matmul emit softmax denominators.  PV trails the exp stream by one
    jg so the PE never head-of-line blocks on an exp result.
  - GPSIMD cannot touch PSUM, so PSUM->SBUF casts go to ACT (K/Q/V)
    and DVE (prob divides, residual); Pool keeps the SBUF-only work
    (partition broadcasts, LayerNorm normalize, DMAs).  The whole
    (pair, itile, jg) slot sequence is software-pipelined: PVs trail
    the exp stream by two slots across itile boundaries, divide chains
    chase the last PV, and projection / V-production matmul groups for
    upcoming pairs fill the PE slack inside the attention slots.
  - LayerNorm applies (x - mean) * rstd on device; the gamma/beta
    affine is applied on the host after gathering (exact for any
    gamma/beta).  One act-table load (ln+exp set) is emitted up front.
"""

import os
import sys

os.environ.setdefault("JAX_PLATFORMS", "axon")
sys.path.insert(0, "/opt/trn_rl_repo")

import numpy as np
import ml_dtypes

import concourse.bass as bass
import concourse.tile as tile
from concourse import bacc, mybir
from concourse.bass import ts
from concourse.bass_utils import run_bass_kernel_spmd
from concourse.hw_specs import get_activation_tables

N_HEAD = 16
D_MODEL = 1024
D_HEAD = 64
SEQ = 2048
BATCH = 4
EPS = 1e-5
N_CORES = 8

LOCAL = SEQ // 2            # tokens owned per core (1024)
N_PAIR = N_HEAD // 2        # head pairs (8)
CC = D_MODEL // 128         # 128-contraction chunks (8)
CP = CC // 2                # DoubleRow contraction pair chunks (4)
JT = SEQ // 128             # j tiles (16)
JG = JT // 2                # j tile pairs (8)
IB_ALL = SEQ // 512         # 512-token blocks, all tokens (4)
IB_LOC = LOCAL // 512       # 512-token blocks, local tokens (2)

F32 = mybir.dt.float32
F16 = mybir.dt.float16
I16 = mybir.dt.int16
E4 = mybir.dt.float8e4
E5 = mybir.dt.float8e5
AF = mybir.ActivationFunctionType
ALU = mybir.AluOpType
DR = mybir.MatmulPerfMode.DoubleRow

UPS = 32.0                                  # host weight upscale
SCALE_S = 1.0 / (2.0 * UPS * UPS * 8.0)     # exp scale: stride0 x2, x32 q/k, sqrt(64)
INV_OUT = 1.0 / (UPS * UPS)                 # out-proj downscale
A16 = 1024.0 / float(np.log(2.0))           # Schraudolph fp16 slope
C16 = 15360.0 - 45.0                        # Schraudolph fp16 offset

FLEX_MOD = 10      # every FLEX_MOD-th odd-head exp slot runs on ACT
FLEX_PHASE = 5
# Heads whose exp runs on DVE (Schraudolph, fp16 probs): odd heads.
DVE_HEADS = frozenset((1, 3, 5, 7, 9, 11, 13, 15))
# Per half: fp8 heads first, then fp16 heads (host permutes Wv columns).
V_ORDER = [[n for n in range(8) if n not in DVE_HEADS]
           + [n for n in range(8) if n in DVE_HEADS],
           [n for n in range(8, 16) if n not in DVE_HEADS]
           + [n for n in range(8, 16) if n in DVE_HEADS]]
N_V8 = [sum(1 for n in V_ORDER[0] if n not in DVE_HEADS),
        sum(1 for n in V_ORDER[1] if n not in DVE_HEADS)]
VIDX = {}
for _half in range(2):
    for _i, _n in enumerate(V_ORDER[_half]):
        VIDX[_n] = _i if _i < N_V8[_half] else _i - N_V8[_half]


def build_program():
    nc = bacc.Bacc()

    hb = nc.declare_dram_parameter("hb", [LOCAL, D_MODEL], F32, isOutput=False)
    hbt_d = nc.declare_dram_parameter("hbt", [512, 2 * SEQ], E4, isOutput=False)
    wq = nc.declare_dram_parameter("wq", [512, 2 * D_MODEL], E4, isOutput=False)
    wk = nc.declare_dram_parameter("wk", [512, 2 * D_MODEL], E4, isOutput=False)
    wv = nc.declare_dram_parameter("wv", [512, 2 * D_MODEL], E4, isOutput=False)
    wo = nc.declare_dram_parameter("wo", [512, 2 * D_MODEL], E4, isOutput=False)
    out = nc.declare_dram_parameter("out", [LOCAL, D_MODEL], F32, isOutput=True)

    with tile.TileContext(nc) as tc:
        with (
            tc.tile_pool(name="consts", bufs=1) as consts,
            tc.tile_pool(name="wo_w", bufs=1) as wo_pool,
            tc.tile_pool(name="hbt", bufs=1) as hbt_pool,
            tc.tile_pool(name="w_qk", bufs=1) as wqk_pool,
            tc.tile_pool(name="w_v", bufs=1) as wv_pool,
            tc.tile_pool(name="vsb", bufs=1) as v_pool,
            tc.tile_pool(name="ktq", bufs=3) as ktq_pool,
            tc.tile_pool(name="attnT", bufs=1) as attn_pool,
            tc.tile_pool(name="exp5", bufs=11) as e5_pool,
            tc.tile_pool(name="exp16", bufs=10) as e16_pool,
            tc.tile_pool(name="small", bufs=4) as rec_pool,
            tc.tile_pool(name="xstage", bufs=3) as x_pool,
            tc.tile_pool(name="hbres", bufs=1) as hbr_pool,
            tc.tile_pool(name="stage", bufs=6) as stg_pool,
            tc.tile_pool(name="psum1", bufs=1, space="PSUM") as psum1,
            tc.tile_pool(name="psum3", bufs=3, space="PSUM") as psum2,
        ):
            _emit(nc, tc, hb, hbt_d, wq, wk, wv, wo, out,
                  consts, wo_pool, hbt_pool, wqk_pool, wv_pool, v_pool,
                  ktq_pool, attn_pool, e5_pool, e16_pool, rec_pool, x_pool,
                  hbr_pool, stg_pool, psum1, psum2)

    nc.finalize()
    return nc


def _emit(nc, tc, hb, hbt_d, wq, wk, wv, wo, out,
          consts, wo_pool, hbt_pool, wqk_pool, wv_pool, v_pool,
          ktq_pool, attn_pool, e5_pool, e16_pool, rec_pool, x_pool,
          hbr_pool, stg_pool, psum1, psum2):
    # ---- one act-table load covering Exp + Ln (avoids reload churn) ----
    tables = get_activation_tables(nc.m.arch)
    set_id = list(tables).index("natural_log_exp_and_others")
    nc.scalar.add_instruction(mybir.InstLoadActFuncSet(
        name=nc.get_next_instruction_name(), ins=[], outs=[],
        act_func_set_id=set_id))

    # ---- constants ----
    eps_t = consts.tile([128, 1], F32)
    nc.vector.memset(eps_t[:], EPS)

    # ---- SBUF weight tiles (DoubleRow k-pair interleaved) ----
    wo_sb = [wo_pool.tile([128, 2, D_MODEL], E4, tag=f"wo{c}", name=f"wo{c}")
             for c in range(CP)]
    wq_sb = [wqk_pool.tile([128, 2, D_MODEL], E4, tag=f"wq{c}", name=f"wq{c}")
             for c in range(CP)]
    wk_sb = [wqk_pool.tile([128, 2, D_MODEL], E4, tag=f"wk{c}", name=f"wk{c}")
             for c in range(CP)]

    # V tiles: [128 j, 16 jc, nh, 65] with ones column at d=64; fp8 heads
    # first then fp16 heads per half (host permutes Wv columns to match).
    v8 = [v_pool.tile([128, N_V8[hf], JT, 128], E4, tag=f"v8_{hf}", name=f"v8_{hf}")
          for hf in range(2)]
    v16 = [v_pool.tile([128, max(1, 8 - N_V8[hf]), JT, 68], F16,
                       tag=f"v16_{hf}", name=f"v16_{hf}")
           for hf in range(2)]
    for t in (*v8, *v16):
        nc.vector.memset(t[:, :, :, 64:65], 1.0)

    # ---- h^T fp8 DR tiles + residual rows (preloaded) ----
    hbt = [hbt_pool.tile([128, 2, SEQ], E4, tag=f"hbt{c}", name=f"hbt{c}")
           for c in range(CP)]
    for c, eng in zip(range(CP), (nc.sync, nc.gpsimd, nc.scalar, nc.sync)):
        eng.dma_start(hbt[c][:], hbt_d[ts(c, 128), :])
    hbres = [hbr_pool.tile([128, D_MODEL], F32, tag=f"hbres{i}", name=f"hbres{i}")
             for i in range(8)]
    for i in range(8):
        nc.sync.dma_start(hbres[i][:], hb[ts(i, 128), :])

    def v_dma(half):
        wv_sb = [wv_pool.tile([128, 2, 512], E4, tag=f"wv{half}_{c}",
                              name=f"wv{half}_{c}")
                 for c in range(CP)]
        for c in range(CP):
            nc.gpsimd.dma_start(
                wv_sb[c][:],
                wv[ts(c, 128), 2 * 512 * half: 2 * 512 * (half + 1)])
        return wv_sb

    def v_group(half, wv_sb, j):
        """Produce V (+ones) for one j-chunk of heads 8*half..8*half+7."""
        n8 = N_V8[half]
        ps = psum2.tile([128, 1024], F32, tag="s2", name="vps")[:, 0:512]
        for c in range(CP):
            nc.tensor.matmul(
                ps[:], hbt[c][:, :, ts(j, 128)], wv_sb[c][:],
                start=(c == 0), stop=(c == CP - 1), perf_mode=DR,
            )
        src = ps[:].rearrange("p (n d) -> p n d", n=8)
        nc.scalar.copy(
            v8[half][:, :, j:j + 1, 0:64].squeeze(2), src[:, 0:n8, :])
        if n8 < 8:
            nc.scalar.copy(
                v16[half][:, :, j:j + 1, 0:64].squeeze(2), src[:, n8:8, :])

    # attn output, DoubleRow pair-interleaved for the output projection:
    # at[(pp, itile)] = [128 nd-in-pair, 4 isub, 2 pair-slot, 128 i] fp8
    at = {}
    for pp in range(N_PAIR // 2):
        for itile in range(IB_LOC):
            at[(pp, itile)] = attn_pool.tile(
                [128, 4, 2, 128], E4, tag=f"at{pp}_{itile}", name=f"at{pp}_{itile}")

    def wo_block(itile, pool_heavy=False):
        for s4 in range(4):
            wo_isub(itile, s4, pool_heavy)

    def wo_isub(itile, s4, pool_heavy=False):
        if True:
            isub = 4 * itile + s4
            x = x_pool.tile([128, D_MODEL], F32, tag="x", name="x")
            opsb = psum2.tile([128, 1024], F32, tag="s2", name="ops")
            for dm in range(2):
                ops = opsb[:, ts(dm, 512)]
                for pp in range(N_PAIR // 2):
                    nc.tensor.matmul(
                        ops, at[(pp, itile)][:, s4:s4 + 1, :, :].squeeze(1),
                        wo_sb[pp][:, :, ts(dm, 512)],
                        start=(pp == 0), stop=(pp == N_PAIR // 2 - 1),
                        perf_mode=DR,
                    )
                # x = ops * (1/1024) + hbres   (undo weight upscale)
                nc.vector.scalar_tensor_tensor(
                    x[:, ts(dm, 512)], ops, INV_OUT,
                    hbres[isub][:, ts(dm, 512)],
                    op0=ALU.mult, op1=ALU.add,
                )
            stats = rec_pool.tile([128, 2, 6], F32, tag="bnst", name="st")
            mv = rec_pool.tile([128, 2], F32, tag="bnmv", name="mv")
            for g in range(2):
                nc.vector.bn_stats(stats[:, g, :], x[:, ts(g, 512)])
            nc.vector.bn_aggr(mv[:], stats[:])
            rstd = rec_pool.tile([128, 1], F32, tag="rstd", name="rstd")
            nc.scalar.activation(rstd[:], mv[:, 1:2], AF.Ln, bias=eps_t[:])
            nc.scalar.activation(rstd[:], rstd[:], AF.Exp, scale=-0.5)
            y = x_pool.tile([128, D_MODEL], F32, tag="y", name="y")
            nc.gpsimd.tensor_scalar(
                y[:], x[:], mv[:, 0:1], rstd[:],
                op0=ALU.subtract, op1=ALU.mult,
            )
            nc.sync.dma_start(out[ts(isub, 128), :], y[:])

    def recip_of(acc, h):
        rec = rec_pool.tile([1, 512], F32, tag="rec", name="rec")
        nc.vector.reciprocal(rec[:], acc[h][64:65, :])
        return rec

    def divide(p, itile, acc, h, rec):
        """Normalize acc -> at (broadcast on Pool, divide DVE/Pool)."""
        rb = rec_pool.tile([64, 512], F32, tag="recb", name="rb")
        nc.gpsimd.partition_broadcast(rb[:], rec[:])
        pp, slot = divmod(p, 2)
        dst = at[(pp, itile)][ts(h, 64), :, slot:slot + 1, :].squeeze(2)
        nc.vector.tensor_tensor(
            dst,
            acc[h][0:64, :].rearrange("p (a b) -> p a b", a=4),
            rb[:].rearrange("p (a b) -> p a b", a=4),
            op=ALU.mult,
        )

    def kq_group(kt_p, qt_p, p, g):
        """One projection matmul group for pair p (g<4: K j-block, else Q)."""
        if g < IB_ALL:
            ps = psum2.tile([128, 1024], F32, tag="s2", name="kps")[:, 0:512]
            for c in range(CP):
                nc.tensor.matmul(
                    ps[:], wk_sb[c][:, :, ts(p, 128)], hbt[c][:, :, ts(g, 512)],
                    start=(c == 0), stop=(c == CP - 1), perf_mode=DR,
                )
            nc.scalar.copy(kt_p[:, ts(g, 512)], ps[:])
        else:
            ib = g - IB_ALL
            ps = psum2.tile([128, 1024], F32, tag="s2", name="qps")[:, 0:512]
            for c in range(CP):
                nc.tensor.matmul(
                    ps[:], wq_sb[c][:, :, ts(p, 128)], hbt[c][:, :, ts(ib, 512)],
                    start=(c == 0), stop=(c == CP - 1), perf_mode=DR,
                )
            nc.scalar.copy(qt_p[:, ts(ib, 512)], ps[:])

    ktq = {}  # pair -> (kt, qt) tiles (ring of 2)

    def new_ktq(p):
        ktq[p] = (ktq_pool.tile([128, SEQ], E4, tag="kt", name="kt_p"),
                  ktq_pool.tile([128, LOCAL], E4, tag="qt", name="qt_p"))
        return ktq[p]

    fillers = []  # deferred emission closures, popped 2 per jg slot

    # pair 0 prologue: weights, own projections up front, V fillers.
    # wq first and Q-projection before K: the first scores need qt(it0)
    # plus only the first K j-block.
    for c in range(CP):
        nc.gpsimd.dma_start(wq_sb[c][:], wq[ts(c, 128), :])
        nc.scalar.dma_start(wk_sb[c][:], wk[ts(c, 128), :])
        nc.sync.dma_start(wo_sb[c][:], wo[ts(c, 128), :])
    kt0, qt0 = new_ktq(0)
    for g in (IB_ALL, 0, IB_ALL + 1, 1, 2, 3):
        kq_group(kt0, qt0, 0, g)
    wv0 = v_dma(0)
    fillers += [(lambda half=0, sb=wv0, j=j: v_group(half, sb, j))
                for j in range(JT)]

    def pv(p, h, jg, acc, es):
        half = p // 4
        n = 2 * p + h
        e = es[(jg, h)]
        if n in DVE_HEADS:
            ef = e[:].bitcast(F16) if e.dtype == I16 else e[:]
            for u in range(2):
                jc = 2 * jg + u
                nc.tensor.matmul(
                    acc[h][0:65, :],
                    v16[half][:, VIDX[n]:VIDX[n] + 1, jc:jc + 1, 0:65]
                    .squeeze(1).squeeze(1),
                    ef[:, ts(u, 512)],
                    start=(jg == 0 and u == 0),
                    stop=(jg == JG - 1 and u == 1),
                )
        else:
            nc.tensor.matmul(
                acc[h][:],
                v8[half][:, VIDX[n]:VIDX[n] + 1, 2 * jg:2 * jg + 2, :]
                .squeeze(1),
                e[:].rearrange("p (a b) -> p a b", a=2),
                start=(jg == 0), stop=(jg == JG - 1),
                perf_mode=DR,
            )

    # Global slot pipeline: per (p, itile, jg) slot emit scores+exp, pop
    # one trailing PV (previous slot's, possibly across itile/pair
    # boundaries), then fillers.  When an itile's last PV retires, its
    # divide chain (and wo_block(0) for pair 7 itile 0) follows at once.
    pvq = []     # FIFO of (closure, tail_action or None)

    def drain_one():
        if pvq:
            clo, tail = pvq.pop(0)
            clo()
            if tail is not None:
                tail()

    for p in range(N_PAIR):
        if p == 3:
            wv1 = v_dma(1)
            fillers += [(lambda half=1, sb=wv1, j=j: v_group(half, sb, j))
                        for j in range(JT)]

        kt_p, qt_p = ktq.pop(p)

        for itile in range(IB_LOC):
            if itile == 0 and p < N_PAIR - 1:
                ktn, qtn = new_ktq(p + 1)
                fillers += [(lambda k=ktn, q=qtn, pn=p + 1, g=g:
                             kq_group(k, q, pn, g))
                            for g in range(IB_ALL + IB_LOC)]
            acc = [psum1.tile([128, 512], F32, tag=f"acc{h}", name="acc")
                   for h in range(2)]
            es = {}
            for jg in range(JG):
                for h in range(2):
                    s2 = psum2.tile([128, 1024], F32, tag="s2", name="s2")
                    for u in range(2):
                        jc = 2 * jg + u
                        lhsT = (kt_p[ts(h, 64), ts(jc, 128)]
                                .unsqueeze(1).broadcast_to([64, 2, 128]))
                        rhs = (qt_p[ts(h, 64), ts(itile, 512)]
                               .unsqueeze(1).broadcast_to([64, 2, 512]))
                        nc.tensor.matmul(
                            s2[:, ts(u, 512)], lhsT, rhs,
                            start=True, stop=True, perf_mode=DR,
                        )
                    if (2 * p + h) in DVE_HEADS:
                        slot = (2 * p + itile) * 8 + jg
                        if slot % FLEX_MOD == FLEX_PHASE:
                            # flex this odd-head exp onto ACT (fp16 out,
                            # same fp16 PV path) to balance engine load
                            e = e16_pool.tile([128, 1024], F16, tag="e16",
                                              name="e16")
                            nc.scalar.activation(e[:], s2[:], AF.Exp,
                                                 scale=SCALE_S)
                        else:
                            e = e16_pool.tile([128, 1024], I16, tag="e16",
                                              name="e16")
                            nc.vector.tensor_scalar(
                                e[:], s2[:], A16 * SCALE_S, C16,
                                op0=ALU.mult, op1=ALU.add,
                            )
                    else:
                        e = e5_pool.tile([128, 1024], E5, tag="e5", name="e5")
                        nc.scalar.activation(e[:], s2[:], AF.Exp, scale=SCALE_S)
                    es[(jg, h)] = e

                def tail(p=p, itile=itile, acc=acc, es=es):
                    recs = [recip_of(acc, h) for h in range(2)]
                    for h in range(2):
                        divide(p, itile, acc, h, recs[h])
                    if p == N_PAIR - 1 and itile == 0:
                        for s4 in range(4):
                            fillers.extend(
                                [(lambda s4=s4: wo_isub(0, s4,
                                                        pool_heavy=True)),
                                 (lambda: None), (lambda: None),
                                 (lambda: None)])

                if len(pvq) > 3:
                    drain_one()
                pvq.append((
                    (lambda p=p, jg=jg, acc=acc, es=es:
                     [pv(p, h, jg, acc, es) for h in range(2)]),
                    tail if jg == JG - 1 else None,
                ))
                for _ in range(2):
                    if fillers:
                        fillers.pop(0)()
    while pvq:
        drain_one()
    while fillers:
        fillers.pop(0)()
    wo_block(1)


_program_cache = {}


def _get_program():
    if "nc" not in _program_cache:
        _program_cache["nc"] = build_program()
    return _program_cache["nc"]


def _interleave_k(w):
    """[1024, C] -> [512, 2C] DoubleRow k-pair layout: out[128*cp + k,
    t*C + c] = w[256*cp + 128*t + k, c]."""
    C = w.shape[1]
    return np.ascontiguousarray(
        w.reshape(4, 2, 128, C).transpose(0, 2, 1, 3).reshape(512, 2 * C))


def _shard_inputs(h, Wq, Wkv, Wo):
    """Build the 8 per-core input maps (host-side numpy only)."""
    h = np.asarray(h, np.float32)
    Wq = np.asarray(Wq, np.float32)
    Wkv = np.asarray(Wkv, np.float32)
    Wo = np.asarray(Wo, np.float32)

    E4n = ml_dtypes.float8_e4m3
    Wq8 = _interleave_k((Wq * UPS).astype(E4n))
    Wk8 = _interleave_k((Wkv[:, :N_HEAD * D_HEAD] * UPS).astype(E4n))
    Wv = Wkv[:, N_HEAD * D_HEAD:] * UPS
    # permute V columns: per half, fp8 heads first then fp16 heads
    perm = [n * 64 + d for hf in range(2) for n in V_ORDER[hf]
            for d in range(64)]
    Wvp = np.ascontiguousarray(Wv[:, perm]).astype(E4n)
    # interleave each 512-column half separately so the per-half DMA
    # slice is a valid [128, 2, 512] DoubleRow tile
    Wv8 = np.concatenate(
        [_interleave_k(Wvp[:, 512 * hf:512 * (hf + 1)]) for hf in range(2)],
        axis=1)
    Wo8 = _interleave_k((Wo * UPS).astype(E4n))

    in_maps = []
    for core in range(N_CORES):
        b, r = divmod(core, 2)
        hb_full = h[:, b, :]  # [2048, 1024]
        if r == 0:
            hb_perm = hb_full
        else:
            hb_perm = np.concatenate([hb_full[LOCAL:], hb_full[:LOCAL]], axis=0)
        hbt8 = _interleave_k(np.ascontiguousarray(hb_perm.T).astype(E4n))
        in_maps.append({
            "hb": np.ascontiguousarray(hb_perm[:LOCAL]),
            "hbt": hbt8,
            "wq": Wq8, "wk": Wk8, "wv": Wv8, "wo": Wo8,
        })
    return in_maps


def kernel(h, Wq, Wkv, Wo, gamma, beta, _trace=False):
    nc = _get_program()
    in_maps = _shard_inputs(h, Wq, Wkv, Wo)
    res = run_bass_kernel_spmd(nc, in_maps, list(range(N_CORES)), trace=_trace)
    if _trace:
        kernel.last_results = res

    gamma = np.asarray(gamma, np.float32)
    beta = np.asarray(beta, np.float32)
    out = np.empty((SEQ, BATCH, D_MODEL), np.float32)
    for core in range(N_CORES):
        b, r = divmod(core, 2)
        out[r * LOCAL:(r + 1) * LOCAL, b, :] = res.results[core]["out"]
    # LayerNorm gamma/beta affine (device emits (x - mean) * rstd)
    out *= gamma
    out += beta
    return out
